# revision 11
# baseline (speedup 1.0000x reference)
"""BitNet Transformer MLP on 8 Trainium2 NeuronCores.

Math (per reference):
  sw1 = max(mean|W1|, EPS); wq1 = clip(round(W1/sw1), -1, 1)
  sx[t] = max(max_h|x[t,h]|, EPS)/127; xq = round(x/sx)      (ints in [-127,127])
  h = gelu((xq @ wq1.T) * sx * sw1)                           (exact erf gelu)
  sh[t] = max(max_i|h[t,i]|, EPS)/127; hq = round(h/sh)
  out = (hq @ wq2.T) * sh * sw2

Sharding (tensor-parallel over the intermediate dim I):
  - tokens T flattened; core c quantizes its T/8 token slice in transposed
    layout, AllGather -> xqT (bf16, exact), chunk-interleaved by rank
  - core c holds W1 rows [c*I/8:(c+1)*I/8] and W2 cols [same I-slice];
    host feeds shards PRE-TRANSPOSED: w1t=[H, I/8], w2t=[I/8, H], xt=[H, T/8]
  - per-tensor weight scales via two 1-float AllReduces (W1 first so fc1's
    weight quantization starts before W2 is even scanned)
  - ternary weights are held in SBUF as fp8e4 (exact for {-1,0,1}); both
    weight matrices stay resident (64 KB/partition each)
  - fc1 computes h.T slice [I/8, T] (PE contracts H; lhsT=weights fp8,
    rhs=xq bf16 -- mixed-dtype matmul is exact here), spills h.T as fp16
  - per-token max|h| partials -> one AllReduce(max) of [T]
  - fc2 uses SWAPPED operands: lhsT=hq.T slice (stationary bf16), rhs=w2q
    (moving fp8) -> partial out [tokens, H]; the per-token scale sh*sw2/127
    is a per-PARTITION scalar folded into the PSUM->SBUF copy, and
    ReduceScatter(add) over the token axis hands core c final tokens
    [blk*nb + c*nb/8 ...) per block -- no post-scale pass, no transpose.
  - host interleaves the 8 token-sharded outputs and casts fp16->f32.

All matmuls are exact: quantized activations are integers <=127 (bf16
exact) and weights are ternary (fp8e4 exact); accumulation is fp32 PSUM.
"""

import numpy as np

import concourse.bass as bass
import concourse.mybir as mybir
import concourse.tile as tile
from concourse import bass_utils, bacc

F32 = mybir.dt.float32
BF16 = mybir.dt.bfloat16
FP16 = mybir.dt.float16
FP8 = mybir.dt.float8e4
MAGIC = 12582912.0  # 1.5*2^23: (v+MAGIC)-MAGIC == round-to-nearest-even, |v|<2^22
EPS = 1e-5
Alu = mybir.AluOpType
Act = mybir.ActivationFunctionType

# full problem config
B, S, H, I = 4, 2048, 4096, 16384
T = B * S
NCORES = 8


def build_program(T=T, H=H, I=I, ncores=NCORES, nb=512):
    TS = T // ncores          # token shard (quant phase)
    IS = I // ncores          # I shard per core
    NBLK = T // nb            # token blocks
    KH = H // 128             # fc1 contraction tiles
    KI = IS // 128            # fc2 contraction tiles
    MI = IS // 128            # fc1 m-tiles (h.T rows / 128)
    NHS = H // nb             # fc2 H column slices
    NTS = nb // 128           # fc2 token sub-tiles per block
    TPB = nb // ncores        # tokens per rank per block (RS shard)
    CH = 512                  # phase-Q free-dim chunk
    BPC = TS // nb            # token blocks per AG rank-chunk
    NT32 = nb // 32
    assert nb % 128 == 0 and TS % nb == 0 and IS % 128 == 0 and nb % ncores == 0

    nc = bacc.Bacc("TRN2", target_bir_lowering=False, debug=False, num_devices=ncores)

    x_e = nc.dram_tensor("x", [TS, H], F32, kind="ExternalInput")
    xt_e = nc.dram_tensor("xt", [H, TS], F32, kind="ExternalInput")
    w1t_e = nc.dram_tensor("w1t", [H, IS], F32, kind="ExternalInput")
    w2t_e = nc.dram_tensor("w2t", [IS, H], F32, kind="ExternalInput")
    out_e = nc.dram_tensor("out_h", [NBLK * TPB, H], FP16, kind="ExternalOutput")

    rg = [list(range(ncores))]

    with tile.TileContext(nc) as tc:
        with (
            tc.tile_pool(name="singles", bufs=1) as singles,
            tc.tile_pool(name="wpool", bufs=1) as wpool,
            tc.tile_pool(name="xqp", bufs=4) as xqp,
            tc.tile_pool(name="hlp", bufs=3) as hlp,
            tc.tile_pool(name="hqp", bufs=1) as hqp,
            tc.tile_pool(name="workA", bufs=2) as workA,
            tc.tile_pool(name="workB", bufs=2) as workB,
            tc.tile_pool(name="small", bufs=2) as small,
            tc.tile_pool(name="psum", bufs=4, space="PSUM") as psum,
            tc.tile_pool(name="psbc", bufs=2, space="PSUM") as psbc,
            tc.tile_pool(name="psm2", bufs=2, space="PSUM") as psm2,
            tc.tile_pool(name="dram", bufs=1, space="DRAM") as dram,
        ):
            # ---------------- DRAM scratch ----------------
            sx_ag_in = dram.tile([TS], F32, name="sx_ag_in")
            sx_full = dram.tile([T], F32, name="sx_full", addr_space="Shared")
            xq_ag_in = dram.tile([H, TS], BF16, name="xq_ag_in")
            xqT_full = dram.tile([ncores * H, TS], BF16, name="xqT_full",
                                 addr_space="Shared")
            ws1_in = dram.tile([1, 1], F32, name="ws1_in")
            ws1_out = dram.tile([1, 1], F32, name="ws1_out", addr_space="Shared")
            ws2_in = dram.tile([1, 1], F32, name="ws2_in")
            ws2_out = dram.tile([1, 1], F32, name="ws2_out", addr_space="Shared")
            w2q_dram = dram.tile([IS, H], FP8, name="w2q_dram")
            h_dram = dram.tile([IS, T], FP16, name="h_dram")
            hmax_in = dram.tile([T], F32, name="hmax_in")
            hmax_out = dram.tile([T], F32, name="hmax_out", addr_space="Shared")
            rs_in = [dram.tile([nb, H], FP16, name=f"rs_in_{j}") for j in range(NBLK)]
            rs_out = [dram.tile([TPB, H], FP16, name=f"rs_out_{j}")
                      for j in range(NBLK)]

            # ---------------- constants ----------------
            ones_row = singles.tile([1, 128], F32, name="ones_row")
            nc.any.memset(ones_row[:], 1.0)
            ones_col = singles.tile([128, 1], F32, name="ones_col")
            nc.any.memset(ones_col[:], 1.0)

            # ---------------- phase Q: x row maxima (own token shard) --------
            for it in range(TS // 128):
                rowmax = small.tile([128, 1], F32, tag="rmax")
                for ch, c0 in enumerate(range(0, H, CH)):
                    xtile = workA.tile([128, CH], F32, tag="wa")
                    nc.sync.dma_start(xtile[:], x_e[it * 128:(it + 1) * 128,
                                                    c0:c0 + CH])
                    part = small.tile([128, 1], F32, tag="part")
                    nc.vector.tensor_reduce(part[:], xtile[:], axis=mybir.AxisListType.X,
                                            op=Alu.max, apply_absolute_value=True)
                    if ch == 0:
                        nc.vector.tensor_scalar_max(rowmax[:], part[:], EPS)
                    else:
                        nc.vector.tensor_tensor(rowmax[:], rowmax[:], part[:], Alu.max)
                nc.sync.dma_start(
                    sx_ag_in[it * 128:(it + 1) * 128].rearrange("(p a) -> p a", a=1),
                    rowmax[:])

            nc.gpsimd.collective_compute(
                "AllGather", Alu.bypass, replica_groups=rg,
                ins=[sx_ag_in[:].opt()], outs=[sx_full[:].opt()])

            # ---------------- phase Q: W1 |.| partial sum -> AR1 ----------------
            def abs_sum(src, rows, cols, dst_dram, tag):
                acc = singles.tile([128, 1], F32, name=f"acc_{tag}")
                first = True
                for it in range(rows // 128):
                    for c0 in range(0, cols, 1024):
                        wt = workB.tile([128, 1024], F32, tag="wb")
                        nc.sync.dma_start(wt[:], src[it * 128:(it + 1) * 128,
                                                     c0:c0 + 1024])
                        part = small.tile([128, 1], F32, tag="part")
                        nc.vector.tensor_reduce(part[:], wt[:], axis=mybir.AxisListType.X,
                                                op=Alu.add, apply_absolute_value=True)
                        if first:
                            nc.vector.tensor_copy(acc[:], part[:])
                            first = False
                        else:
                            nc.vector.tensor_tensor(acc[:], acc[:], part[:], Alu.add)
                ps_f = psm2.tile([128, 4], F32, tag="psm2")
                pss = ps_f[0:1, 0:1]
                nc.tensor.matmul(pss, lhsT=acc[:], rhs=ones_col[:], start=True, stop=True)
                ssb = small.tile([1, 1], F32, tag="ssb")
                nc.vector.tensor_copy(ssb[:], pss)
                nc.sync.dma_start(dst_dram[:, :], ssb[:])

            abs_sum(w1t_e, H, IS, ws1_in, "w1")
            nc.gpsimd.collective_compute(
                "AllReduce", Alu.add, replica_groups=rg,
                ins=[ws1_in[:].opt()], outs=[ws1_out[:].opt()])

            # ---------------- phase Q: quantize x (transposed layout) --------
            for c0 in range(0, TS, CH):
                rq_row = small.tile([1, CH], F32, tag="srow")
                nc.sync.dma_start(rq_row[:],
                                  sx_ag_in[c0:c0 + CH].rearrange("(a f) -> a f", a=1))
                nc.vector.reciprocal(rq_row[:], rq_row[:])
                nc.vector.tensor_scalar_mul(rq_row[:], rq_row[:], 127.0)
                ps = psbc.tile([128, CH], F32, tag="psbc")
                nc.tensor.matmul(ps[:], lhsT=ones_row[:], rhs=rq_row[:],
                                 start=True, stop=True)
                rq_bc = small.tile([128, CH], F32, tag="bc")
                nc.vector.tensor_copy(rq_bc[:], ps[:])
                for it in range(H // 128):
                    xtile = workA.tile([128, CH], F32, tag="wa")
                    nc.sync.dma_start(xtile[:, :CH],
                                      xt_e[it * 128:(it + 1) * 128, c0:c0 + CH])
                    nc.vector.tensor_tensor(xtile[:, :CH], xtile[:, :CH], rq_bc[:],
                                            Alu.mult)
                    xqt = workA.tile([128, CH], BF16, tag="xbf")
                    nc.vector.tensor_scalar(xqt[:, :CH], xtile[:, :CH], MAGIC, MAGIC,
                                            Alu.add, Alu.subtract)
                    nc.sync.dma_start(xq_ag_in[it * 128:(it + 1) * 128, c0:c0 + CH],
                                      xqt[:, :CH])

            nc.gpsimd.collective_compute(
                "AllGather", Alu.bypass, replica_groups=rg,
                ins=[xq_ag_in[:].opt()], outs=[xqT_full[:].opt()])

            # ---------------- scale scalars + broadcast columns ----------------
            def bcast_scalar(src_ap, name):
                ps_f = psm2.tile([128, 4], F32, tag="psm2")
                ps = ps_f[:, 0:1]
                nc.tensor.matmul(ps, lhsT=ones_row[:], rhs=src_ap, start=True, stop=True)
                t = singles.tile([128, 1], F32, name=name)
                nc.vector.tensor_copy(t[:], ps)
                return t

            def weight_scale(ws_out_dram, denom, tag):
                sw_sb = singles.tile([1, 1], F32, name=f"sw_{tag}")
                nc.sync.dma_start(sw_sb[:], ws_out_dram[:, :])
                nc.vector.tensor_scalar(sw_sb[:], sw_sb[:], 1.0 / denom, EPS,
                                        Alu.mult, Alu.max)
                rsw_sb = singles.tile([1, 1], F32, name=f"rsw_{tag}")
                nc.vector.reciprocal(rsw_sb[:], sw_sb[:])
                return sw_sb, rsw_sb

            sw1_sb, rsw1_sb = weight_scale(ws1_out, float(I) * H, "w1")
            rsw1_col = bcast_scalar(rsw1_sb[:], "rsw1_col")
            sw1_127_col = bcast_scalar(sw1_sb[:], "sw1_127_col")
            nc.vector.tensor_scalar_mul(sw1_127_col[:], sw1_127_col[:], 1.0 / 127.0)

            # ---------------- quantize W1 -> fp8 resident in SBUF ----------------
            w1sb = wpool.tile([128, KH, IS], FP8, tag="w")
            for k in range(KH):
                for c0 in range(0, IS, 1024):
                    wt = workB.tile([128, 1024], F32, tag="wb")
                    nc.sync.dma_start(wt[:], w1t_e[k * 128:(k + 1) * 128, c0:c0 + 1024])
                    nc.scalar.mul(wt[:], wt[:], rsw1_col[:])
                    nc.vector.tensor_scalar(wt[:], wt[:], MAGIC, MAGIC,
                                            Alu.add, Alu.subtract)
                    nc.vector.tensor_scalar(w1sb[:, k, c0:c0 + 1024], wt[:], 1.0, -1.0,
                                            Alu.min, Alu.max)

            # ---------------- W2 scan/AR/quantize (overlaps fc1) ----------------
            abs_sum(w2t_e, IS, H, ws2_in, "w2")
            nc.gpsimd.collective_compute(
                "AllReduce", Alu.add, replica_groups=rg,
                ins=[ws2_in[:].opt()], outs=[ws2_out[:].opt()])
            sw2_sb, rsw2_sb = weight_scale(ws2_out, float(I) * H, "w2")
            rsw2_col = bcast_scalar(rsw2_sb[:], "rsw2_col")
            sw2_127_sb = singles.tile([1, 1], F32, name="sw2_127_sb")
            nc.vector.tensor_scalar_mul(sw2_127_sb[:], sw2_sb[:], 1.0 / 127.0)
            for k in range(KI):
                for c0 in range(0, H, 1024):
                    wt = workB.tile([128, 1024], F32, tag="wb")
                    nc.sync.dma_start(wt[:], w2t_e[k * 128:(k + 1) * 128, c0:c0 + 1024])
                    nc.scalar.mul(wt[:], wt[:], rsw2_col[:])
                    nc.vector.tensor_scalar(wt[:], wt[:], MAGIC, MAGIC,
                                            Alu.add, Alu.subtract)
                    wq8 = workB.tile([128, 1024], FP8, tag="wq8")
                    nc.vector.tensor_scalar(wq8[:], wt[:], 1.0, -1.0, Alu.min, Alu.max)
                    nc.sync.dma_start(w2q_dram[k * 128:(k + 1) * 128, c0:c0 + 1024],
                                      wq8[:])

            # ---------------- fc1: h.T = gelu(w1q.T-contraction) ----------------
            KHQ = KH // 4
            for blk in range(NBLK):
                crk = blk // BPC
                coff = (blk % BPC) * nb
                xq_half = []
                for hf in range(4):
                    xqh = xqp.tile([128, KHQ, nb], BF16, tag="xq")
                    nc.sync.dma_start(
                        xqh[:],
                        xqT_full[crk * H + hf * KHQ * 128: crk * H + (hf + 1) * KHQ * 128,
                                 coff:coff + nb].rearrange("(k p) c -> p k c", p=128))
                    xq_half.append(xqh)

                s_row = small.tile([1, nb], F32, tag="srow")
                nc.sync.dma_start(
                    s_row[:], sx_full[blk * nb:(blk + 1) * nb].rearrange("(a f) -> a f", a=1))
                ps_b = psbc.tile([128, nb], F32, tag="psbc")
                nc.tensor.matmul(ps_b[:], lhsT=ones_row[:], rhs=s_row[:],
                                 start=True, stop=True)
                m1_t = small.tile([128, nb], F32, tag="bc")
                nc.vector.tensor_scalar(m1_t[:], ps_b[:], sw1_127_col[:], None, Alu.mult)

                gmax = small.tile([128, nb], FP16, tag="gmax")
                gq_slab = None
                for m in range(MI):
                    ps = psum.tile([128, nb], F32, tag="ps")
                    for k in range(KH):
                        nc.tensor.matmul(
                            ps[:], lhsT=w1sb[:, k, m * 128:(m + 1) * 128],
                            rhs=xq_half[k // KHQ][:, k % KHQ, :],
                            start=(k == 0), stop=(k == KH - 1))
                    g = workA.tile([128, nb], F32, tag="g")
                    nc.vector.tensor_tensor(g[:], ps[:], m1_t[:], Alu.mult)
                    if m % 4 == 0:
                        gq_slab = workA.tile([128, 4, nb], FP16, tag="gq")
                    nc.scalar.activation(gq_slab[:, m % 4, :], g[:], Act.Gelu)
                    gabs = workA.tile([128, nb], FP16, tag="gabs")
                    nc.scalar.activation(gabs[:], gq_slab[:, m % 4, :], Act.Abs)
                    if m == 0:
                        nc.vector.tensor_copy(gmax[:], gabs[:])
                    else:
                        nc.vector.tensor_tensor(gmax[:], gmax[:], gabs[:], Alu.max)
                    if m % 4 == 3:
                        mg = m // 4
                        nc.sync.dma_start(
                            h_dram[mg * 512:(mg + 1) * 512, blk * nb:(blk + 1) * nb]
                            .rearrange("(mi p) c -> p mi c", p=128),
                            gq_slab[:])
                # fold per-token max over partitions: 128 -> 32, transpose, reduce
                ftmp = small.tile([64, nb], FP16, tag="foldt")
                nc.vector.tensor_copy(ftmp[0:64], gmax[64:128])
                nc.vector.tensor_tensor(gmax[0:64], gmax[0:64], ftmp[0:64], Alu.max)
                nc.vector.tensor_copy(ftmp[0:32], gmax[32:64])
                nc.vector.tensor_tensor(gmax[0:32], gmax[0:32], ftmp[0:32], Alu.max)
                gmt = small.tile([32, nb], FP16, tag="gmt")
                nc.vector.transpose(gmt[:], gmax[0:32, :])
                red = small.tile([32, NT32], F32, tag="red")
                nc.vector.tensor_reduce(
                    red[:], gmt[:].rearrange("p (c q) -> p c q", q=32),
                    axis=mybir.AxisListType.X, op=Alu.max)
                nc.sync.dma_start(
                    hmax_in[blk * nb:(blk + 1) * nb].rearrange("(c p) -> p c", p=32),
                    red[:])

            nc.gpsimd.collective_compute(
                "AllReduce", Alu.max, replica_groups=rg,
                ins=[hmax_in[:].opt()], outs=[hmax_out[:].opt()])

            # ---------------- fc2: out[tok, H] with swapped operands ----------
            w2sb = wpool.tile([128, KI, H], FP8, tag="w")
            nc.sync.dma_start(w2sb[:],
                              w2q_dram[:, :].rearrange("(k p) h -> p k h", p=128))

            KIH = KI // 2
            for blk in range(NBLK):
                hl_half = []
                for hf in range(2):
                    hlh = hlp.tile([128, KIH, nb], FP16, tag="hl")
                    nc.sync.dma_start(
                        hlh[:],
                        h_dram[hf * KIH * 128:(hf + 1) * KIH * 128,
                               blk * nb:(blk + 1) * nb]
                        .rearrange("(k p) c -> p k c", p=128))
                    hl_half.append(hlh)

                s_row = small.tile([1, nb], F32, tag="srow")
                nc.sync.dma_start(
                    s_row[:],
                    hmax_out[blk * nb:(blk + 1) * nb].rearrange("(a f) -> a f", a=1))
                nc.vector.tensor_scalar_max(s_row[:], s_row[:], EPS)
                r_row = small.tile([1, nb], F32, tag="rrow")
                nc.vector.reciprocal(r_row[:], s_row[:])
                ps_b = psbc.tile([128, nb], F32, tag="psbc")
                nc.tensor.matmul(ps_b[:], lhsT=ones_row[:], rhs=r_row[:],
                                 start=True, stop=True)
                rq_t = small.tile([128, nb], F32, tag="bc")
                nc.vector.tensor_scalar_mul(rq_t[:], ps_b[:], 127.0)

                # per-token out-scale columns: m2[tok] = sh*sw2/127 (per-partition)
                ps_m2 = psm2.tile([128, 4], F32, tag="psm2")
                for j in range(NTS):
                    nc.tensor.matmul(ps_m2[:, j:j + 1],
                                     lhsT=s_row[0:1, j * 128:(j + 1) * 128],
                                     rhs=sw2_127_sb[:], start=True, stop=True)
                m2cols = small.tile([128, NTS], F32, tag="m2c")
                nc.vector.tensor_copy(m2cols[:], ps_m2[:, 0:NTS])

                hq = hqp.tile([128, KI, nb], BF16, tag="hq")
                for ki in range(KI):
                    ht = workB.tile([128, nb], F32, tag="ht")
                    nc.vector.tensor_tensor(ht[:], hl_half[ki // KIH][:, ki % KIH, :],
                                            rq_t[:], Alu.mult)
                    nc.vector.tensor_scalar(hq[:, ki, :], ht[:], MAGIC, MAGIC,
                                            Alu.add, Alu.subtract)

                for ts_ in range(NTS):
                    for hs in range(NHS):
                        ps = psum.tile([128, nb], F32, tag="ps")
                        for ki in range(KI):
                            nc.tensor.matmul(
                                ps[:], lhsT=hq[:, ki, ts_ * 128:(ts_ + 1) * 128],
                                rhs=w2sb[:, ki, hs * nb:(hs + 1) * nb],
                                start=(ki == 0), stop=(ki == KI - 1))
                        ot = workA.tile([128, nb], FP16, tag="ot")
                        nc.vector.tensor_scalar(ot[:], ps[:], m2cols[:, ts_:ts_ + 1],
                                                None, Alu.mult)
                        nc.sync.dma_start(
                            rs_in[blk][ts_ * 128:(ts_ + 1) * 128, hs * nb:(hs + 1) * nb],
                            ot[:])
                nc.gpsimd.collective_compute(
                    "ReduceScatter", Alu.add, replica_groups=rg,
                    ins=[rs_in[blk][:].opt()], outs=[rs_out[blk][:].opt()])
                nc.sync.dma_start(out_e[blk * TPB:(blk + 1) * TPB, :], rs_out[blk][:, :])

    nc.compile()
    return nc


_PROGRAM_CACHE = {}


def _get_program(key):
    if key not in _PROGRAM_CACHE:
        _PROGRAM_CACHE[key] = build_program(*key)
    return _PROGRAM_CACHE[key]


def make_in_maps(x, W1, W2, ncores=NCORES):
    t, h = x.reshape(-1, x.shape[-1]).shape
    i = W1.shape[0]
    xf = np.ascontiguousarray(x.reshape(t, h), dtype=np.float32)
    ts, isd = t // ncores, i // ncores
    in_maps = []
    for c in range(ncores):
        xs = xf[c * ts:(c + 1) * ts]
        in_maps.append({
            "x": xs,
            "xt": np.ascontiguousarray(xs.T),
            "w1t": np.ascontiguousarray(W1[c * isd:(c + 1) * isd, :].T, dtype=np.float32),
            "w2t": np.ascontiguousarray(W2[:, c * isd:(c + 1) * isd].T, dtype=np.float32),
        })
    return in_maps


def assemble_out(per_core_out, x_shape, t, h, ncores=NCORES, nb=512):
    """per_core_out[c]: [T/ncores, H] fp16, rows = (blk, tok-in-rank-shard).
    token t = blk*nb + c*(nb/ncores) + r."""
    nblk = t // nb
    tpb = nb // ncores
    stacked = np.stack(per_core_out, axis=0).reshape(ncores, nblk, tpb, h)
    out = stacked.transpose(1, 0, 2, 3).reshape(t, h).astype(np.float32)
    return out.reshape(x_shape)


def run(x, W1, W2, trace=False, trace_kwargs=None):
    """Run the distributed kernel on full inputs. Returns (out, BassKernelResults)."""
    t, h = x.reshape(-1, x.shape[-1]).shape
    i = W1.shape[0]
    nc = _get_program((t, h, i, NCORES))
    in_maps = make_in_maps(x, W1, W2)
    res = bass_utils.run_bass_kernel_spmd(
        nc, in_maps, core_ids=list(range(NCORES)), trace=trace,
        **(trace_kwargs or {}),
    )
    out = assemble_out([res.results[c]["out_h"] for c in range(NCORES)],
                       x.shape, t, h)
    return out, res


def kernel(x, W1, W2):
    out, _ = run(x, W1, W2)
    return out


class TimedRunner:
    """Compile once, keep inputs on device, time repeated executions.

    Mirrors bass2jax.run_bass_via_pjrt's multi-core path but persists the
    device-side inputs so repeat calls measure (dispatch + HW execution)
    only, not the host->device staging.
    """

    def __init__(self, nc, in_maps):
        import jax
        import concourse.mybir as mybir_
        from concourse import bass2jax
        from jax.experimental.shard_map import shard_map
        from jax.sharding import Mesh, PartitionSpec, NamedSharding

        bass2jax.install_neuronx_cc_hook()
        n_cores = len(in_maps)
        partition_name = nc.partition_id_tensor.name if nc.partition_id_tensor else None
        in_names, out_names, out_avals = [], [], []
        for alloc in nc.m.functions[0].allocations:
            if not isinstance(alloc, mybir_.MemoryLocationSet):
                continue
            name = alloc.memorylocations[0].name
            if alloc.kind == "ExternalInput":
                if name != partition_name:
                    in_names.append(name)
            elif alloc.kind == "ExternalOutput":
                out_names.append(name)
                out_avals.append(jax.core.ShapedArray(
                    tuple(alloc.tensor_shape), mybir_.dt.np(alloc.dtype)))
        n_params = len(in_names)
        n_outs = len(out_avals)
        all_in_names = list(in_names) + list(out_names)
        if partition_name is not None:
            all_in_names.append(partition_name)
        donate = tuple(range(n_params, n_params + n_outs))

        def _body(*args):
            operands = list(args)
            if partition_name is not None:
                operands.append(bass2jax.partition_id_tensor())
            outs = bass2jax._bass_exec_p.bind(
                *operands,
                out_avals=tuple(out_avals),
                in_names=tuple(all_in_names),
                out_names=tuple(out_names),
                lowering_input_output_aliases=(),
                sim_require_finite=True,
                sim_require_nnan=True,
                nc=nc,
            )
            return tuple(outs)

        devices = jax.devices()[:n_cores]
        mesh = Mesh(np.asarray(devices), ("core",))
        in_specs = (PartitionSpec("core"),) * (n_params + n_outs)
        out_specs = (PartitionSpec("core"),) * n_outs
        self._fn = jax.jit(
            shard_map(_body, mesh=mesh, in_specs=in_specs, out_specs=out_specs,
                      check_rep=False),
            donate_argnums=donate, keep_unused=True,
        )
        sh = NamedSharding(mesh, PartitionSpec("core"))
        concat_in = [
            np.concatenate([np.asarray(in_maps[c][nm]) for c in range(n_cores)], axis=0)
            for nm in in_names
        ]
        self._dev_in = [jax.device_put(a, sh) for a in concat_in]
        self._zero_shapes = [(n_cores * a.shape[0], *a.shape[1:]) for a in out_avals]
        self._zero_dtypes = [a.dtype for a in out_avals]
        self._sh = sh
        self._jax = jax
        self.out_names = out_names
        self.out_avals = out_avals
        self.n_cores = n_cores

    def run_once(self):
        import time
        jax = self._jax
        zeros = [jax.device_put(np.zeros(s, d), self._sh)
                 for s, d in zip(self._zero_shapes, self._zero_dtypes)]
        jax.block_until_ready(zeros)
        t0 = time.perf_counter()
        outs = self._fn(*self._dev_in, *zeros)
        jax.block_until_ready(outs)
        dt = time.perf_counter() - t0
        results = [
            {nm: np.asarray(outs[i]).reshape(self.n_cores, *self.out_avals[i].shape)[c]
             for i, nm in enumerate(self.out_names)}
            for c in range(self.n_cores)
        ]
        return results, dt


# revision 16
# speedup vs baseline: 1.0384x; 1.0384x over previous
"""BitNet Transformer MLP on 8 Trainium2 NeuronCores.

Math (per reference):
  sw1 = max(mean|W1|, EPS); wq1 = clip(round(W1/sw1), -1, 1)
  sx[t] = max(max_h|x[t,h]|, EPS)/127; xq = round(x/sx)      (ints in [-127,127])
  h = gelu((xq @ wq1.T) * sx * sw1)                           (exact erf gelu)
  sh[t] = max(max_i|h[t,i]|, EPS)/127; hq = round(h/sh)
  out = (hq @ wq2.T) * sh * sw2

Sharding (tensor-parallel over the intermediate dim I):
  - tokens T flattened; core c quantizes its T/8 token slice in transposed
    layout, AllGather -> xqT (bf16, exact), chunk-interleaved by rank
  - core c holds W1 rows [c*I/8:(c+1)*I/8] and W2 cols [same I-slice];
    host feeds shards PRE-TRANSPOSED: w1t=[H, I/8], w2t=[I/8, H], xt=[H, T/8]
  - per-tensor weight scales via two 1-float AllReduces (W1 first so fc1's
    weight quantization starts before W2 is even scanned)
  - ternary weights are held in SBUF as fp8e4 (exact for {-1,0,1}); both
    weight matrices stay resident (64 KB/partition each)
  - fc1 computes h.T slice [I/8, T] (PE contracts H; lhsT=weights fp8,
    rhs=xq bf16 -- mixed-dtype matmul is exact here), spills h.T as fp16
  - per-token max|h| partials -> one AllReduce(max) of [T]
  - fc2 uses SWAPPED operands: lhsT=hq.T slice (stationary bf16), rhs=w2q
    (moving fp8) -> partial out [tokens, H]; the per-token scale sh*sw2/127
    is a per-PARTITION scalar folded into the PSUM->SBUF copy, and
    ReduceScatter(add) over the token axis hands core c final tokens
    [blk*nb + c*nb/8 ...) per block -- no post-scale pass, no transpose.
  - host interleaves the 8 token-sharded outputs and casts fp16->f32.

All matmuls are exact: quantized activations are integers <=127 (bf16
exact) and weights are ternary (fp8e4 exact); accumulation is fp32 PSUM.
"""

import numpy as np

import concourse.bass as bass
import concourse.mybir as mybir
import concourse.tile as tile
from concourse import bass_utils, bacc

F32 = mybir.dt.float32
BF16 = mybir.dt.bfloat16
FP16 = mybir.dt.float16
FP8 = mybir.dt.float8e4
MAGIC = 12582912.0  # 1.5*2^23: (v+MAGIC)-MAGIC == round-to-nearest-even, |v|<2^22
EPS = 1e-5
Alu = mybir.AluOpType
Act = mybir.ActivationFunctionType

# full problem config
B, S, H, I = 4, 2048, 4096, 16384
T = B * S
NCORES = 8


def build_program(T=T, H=H, I=I, ncores=NCORES, nb=512):
    TS = T // ncores          # token shard (quant phase)
    IS = I // ncores          # I shard per core
    NBLK = T // nb            # token blocks
    KH = H // 128             # fc1 contraction tiles
    KI = IS // 128            # fc2 contraction tiles
    MI = IS // 128            # fc1 m-tiles (h.T rows / 128)
    NHS = H // nb             # fc2 H column slices
    NTS = nb // 128           # fc2 token sub-tiles per block
    TPB = nb // ncores        # tokens per rank per block (RS shard)
    CH = 512                  # phase-Q free-dim chunk
    BPC = TS // nb            # token blocks per AG rank-chunk
    NT32 = nb // 32
    assert nb % 128 == 0 and TS % nb == 0 and IS % 128 == 0 and nb % ncores == 0

    nc = bacc.Bacc("TRN2", target_bir_lowering=False, debug=False, num_devices=ncores)

    x_e = nc.dram_tensor("x", [TS, H], F32, kind="ExternalInput")
    xt_e = nc.dram_tensor("xt", [H, TS], F32, kind="ExternalInput")
    w1t_e = nc.dram_tensor("w1t", [H, IS], F32, kind="ExternalInput")
    w2t_e = nc.dram_tensor("w2t", [IS, H], F32, kind="ExternalInput")
    out_e = nc.dram_tensor("out_h", [NBLK * TPB, H], FP16, kind="ExternalOutput")

    rg = [list(range(ncores))]

    with tile.TileContext(nc) as tc:
        with (
            tc.tile_pool(name="singles", bufs=1) as singles,
            tc.tile_pool(name="wpool", bufs=1) as wpool,
            tc.tile_pool(name="xqp", bufs=5) as xqp,
            tc.tile_pool(name="hlp", bufs=2) as hlp,
            tc.tile_pool(name="hqp", bufs=2) as hqp,
            tc.tile_pool(name="workA", bufs=2) as workA,
            tc.tile_pool(name="workB", bufs=2) as workB,
            tc.tile_pool(name="small", bufs=2) as small,
            tc.tile_pool(name="psum", bufs=6, space="PSUM") as psum,
            tc.tile_pool(name="psbc", bufs=2, space="PSUM") as psbc,
            tc.tile_pool(name="dram", bufs=1, space="DRAM") as dram,
        ):
            # ---------------- DRAM scratch ----------------
            sx_ag_in = dram.tile([TS], F32, name="sx_ag_in")
            sx_full = dram.tile([T], F32, name="sx_full", addr_space="Shared")
            xq_ag_in = dram.tile([H, TS], BF16, name="xq_ag_in")
            xqT_full = dram.tile([ncores * H, TS], BF16, name="xqT_full",
                                 addr_space="Shared")
            ws1_in = dram.tile([1, 1], F32, name="ws1_in")
            ws1_out = dram.tile([1, 1], F32, name="ws1_out", addr_space="Shared")
            ws2_in = dram.tile([1, 1], F32, name="ws2_in")
            ws2_out = dram.tile([1, 1], F32, name="ws2_out", addr_space="Shared")
            w2q_dram = dram.tile([IS, H], FP8, name="w2q_dram")
            h_dram = dram.tile([IS, T], FP16, name="h_dram")
            hmax_in = dram.tile([T], F32, name="hmax_in")
            hmax_out = dram.tile([T], F32, name="hmax_out", addr_space="Shared")
            rs_in = [dram.tile([nb, H], FP16, name=f"rs_in_{j}") for j in range(NBLK)]
            rs_out = [dram.tile([TPB, H], FP16, name=f"rs_out_{j}")
                      for j in range(NBLK)]

            # ---------------- constants ----------------
            ones_row = singles.tile([1, 128], F32, name="ones_row")
            nc.any.memset(ones_row[:], 1.0)
            ones_col = singles.tile([128, 1], F32, name="ones_col")
            nc.any.memset(ones_col[:], 1.0)

            # ---------------- phase Q: W1 |.| partial sum -> AR1 ----------------
            def abs_sum(src, rows, cols, dst_dram, tag):
                acc = singles.tile([128, 1], F32, name=f"acc_{tag}")
                first = True
                for it in range(rows // 128):
                    for c0 in range(0, cols, 1024):
                        wt = workB.tile([128, 1024], F32, tag="wb")
                        nc.sync.dma_start(wt[:], src[it * 128:(it + 1) * 128,
                                                     c0:c0 + 1024])
                        part = small.tile([128, 1], F32, tag="part")
                        nc.vector.tensor_reduce(part[:], wt[:], axis=mybir.AxisListType.X,
                                                op=Alu.add, apply_absolute_value=True)
                        if first:
                            nc.vector.tensor_copy(acc[:], part[:])
                            first = False
                        else:
                            nc.vector.tensor_tensor(acc[:], acc[:], part[:], Alu.add)
                ps_f = psbc.tile([128, 4], F32, tag="psbc")
                pss = ps_f[0:1, 0:1]
                nc.tensor.matmul(pss, lhsT=acc[:], rhs=ones_col[:], start=True, stop=True)
                ssb = small.tile([1, 1], F32, tag="ssb")
                nc.vector.tensor_copy(ssb[:], pss)
                nc.sync.dma_start(dst_dram[:, :], ssb[:])

            abs_sum(w1t_e, H, IS, ws1_in, "w1")

            # x row maxima (own token shard); DMAs queue after the w1 scan
            for it in range(TS // 128):
                rowmax = small.tile([128, 1], F32, tag="rmax")
                for ch, c0 in enumerate(range(0, H, CH)):
                    xtile = workA.tile([128, CH], F32, tag="wa")
                    nc.sync.dma_start(xtile[:], x_e[it * 128:(it + 1) * 128,
                                                    c0:c0 + CH])
                    part = small.tile([128, 1], F32, tag="part")
                    nc.vector.tensor_reduce(part[:], xtile[:], axis=mybir.AxisListType.X,
                                            op=Alu.max, apply_absolute_value=True)
                    if ch == 0:
                        nc.vector.tensor_scalar_max(rowmax[:], part[:], EPS)
                    else:
                        nc.vector.tensor_tensor(rowmax[:], rowmax[:], part[:], Alu.max)
                nc.sync.dma_start(
                    sx_ag_in[it * 128:(it + 1) * 128].rearrange("(p a) -> p a", a=1),
                    rowmax[:])

            nc.gpsimd.collective_compute(
                "AllReduce", Alu.add, replica_groups=rg,
                ins=[ws1_in[:].opt()], outs=[ws1_out[:].opt()])
            nc.gpsimd.collective_compute(
                "AllGather", Alu.bypass, replica_groups=rg,
                ins=[sx_ag_in[:].opt()], outs=[sx_full[:].opt()])

            # ---------------- phase Q: quantize x (transposed layout) --------
            for c0 in range(0, TS, CH):
                rq_row = small.tile([1, CH], F32, tag="srow")
                nc.sync.dma_start(rq_row[:],
                                  sx_ag_in[c0:c0 + CH].rearrange("(a f) -> a f", a=1))
                nc.vector.reciprocal(rq_row[:], rq_row[:])
                nc.vector.tensor_scalar_mul(rq_row[:], rq_row[:], 127.0)
                ps = psbc.tile([128, CH], F32, tag="psbc")
                nc.tensor.matmul(ps[:], lhsT=ones_row[:], rhs=rq_row[:],
                                 start=True, stop=True)
                rq_bc = small.tile([128, CH], F32, tag="bc")
                nc.vector.tensor_copy(rq_bc[:], ps[:])
                for it in range(H // 128):
                    xtile = workA.tile([128, CH], F32, tag="wa")
                    nc.sync.dma_start(xtile[:, :CH],
                                      xt_e[it * 128:(it + 1) * 128, c0:c0 + CH])
                    nc.vector.tensor_tensor(xtile[:, :CH], xtile[:, :CH], rq_bc[:],
                                            Alu.mult)
                    xqt = workA.tile([128, CH], BF16, tag="xbf")
                    nc.vector.tensor_scalar(xqt[:, :CH], xtile[:, :CH], MAGIC, MAGIC,
                                            Alu.add, Alu.subtract)
                    nc.sync.dma_start(xq_ag_in[it * 128:(it + 1) * 128, c0:c0 + CH],
                                      xqt[:, :CH])

            nc.gpsimd.collective_compute(
                "AllGather", Alu.bypass, replica_groups=rg,
                ins=[xq_ag_in[:].opt()], outs=[xqT_full[:].opt()])

            # ---------------- scale scalars + broadcast columns ----------------
            def bcast_scalar(src_ap, name):
                ps_f = psbc.tile([128, 4], F32, tag="psbc")
                ps = ps_f[:, 0:1]
                nc.tensor.matmul(ps, lhsT=ones_row[:], rhs=src_ap, start=True, stop=True)
                t = singles.tile([128, 1], F32, name=name)
                nc.vector.tensor_copy(t[:], ps)
                return t

            def weight_scale(ws_out_dram, denom, tag):
                sw_sb = singles.tile([1, 1], F32, name=f"sw_{tag}")
                nc.sync.dma_start(sw_sb[:], ws_out_dram[:, :])
                nc.vector.tensor_scalar(sw_sb[:], sw_sb[:], 1.0 / denom, EPS,
                                        Alu.mult, Alu.max)
                rsw_sb = singles.tile([1, 1], F32, name=f"rsw_{tag}")
                nc.vector.reciprocal(rsw_sb[:], sw_sb[:])
                return sw_sb, rsw_sb

            sw1_sb, rsw1_sb = weight_scale(ws1_out, float(I) * H, "w1")
            rsw1_col = bcast_scalar(rsw1_sb[:], "rsw1_col")
            sw1_127_col = bcast_scalar(sw1_sb[:], "sw1_127_col")
            nc.vector.tensor_scalar_mul(sw1_127_col[:], sw1_127_col[:], 1.0 / 127.0)

            # ---------------- quantize W1 -> fp8 resident in SBUF ----------------
            w1sb = wpool.tile([128, KH, IS], FP8, tag="w")
            for k in range(KH):
                for c0 in range(0, IS, 1024):
                    wt = workB.tile([128, 1024], F32, tag="wb")
                    nc.sync.dma_start(wt[:], w1t_e[k * 128:(k + 1) * 128, c0:c0 + 1024])
                    nc.scalar.mul(wt[:], wt[:], rsw1_col[:])
                    nc.vector.tensor_scalar(wt[:], wt[:], MAGIC, MAGIC,
                                            Alu.add, Alu.subtract)
                    nc.vector.tensor_scalar(w1sb[:, k, c0:c0 + 1024], wt[:], 1.0, -1.0,
                                            Alu.min, Alu.max)

            # ---------------- W2 scan/AR/quantize (overlaps fc1) ----------------
            abs_sum(w2t_e, IS, H, ws2_in, "w2")
            nc.gpsimd.collective_compute(
                "AllReduce", Alu.add, replica_groups=rg,
                ins=[ws2_in[:].opt()], outs=[ws2_out[:].opt()])
            sw2_sb, rsw2_sb = weight_scale(ws2_out, float(I) * H, "w2")
            rsw2_col = bcast_scalar(rsw2_sb[:], "rsw2_col")
            sw2_127_sb = singles.tile([1, 1], F32, name="sw2_127_sb")
            nc.vector.tensor_scalar_mul(sw2_127_sb[:], sw2_sb[:], 1.0 / 127.0)
            for k in range(KI):
                for c0 in range(0, H, 1024):
                    wt = workB.tile([128, 1024], F32, tag="wb")
                    nc.sync.dma_start(wt[:], w2t_e[k * 128:(k + 1) * 128, c0:c0 + 1024])
                    nc.scalar.mul(wt[:], wt[:], rsw2_col[:])
                    nc.vector.tensor_scalar(wt[:], wt[:], MAGIC, MAGIC,
                                            Alu.add, Alu.subtract)
                    wq8 = workB.tile([128, 1024], FP8, tag="wq8")
                    nc.vector.tensor_scalar(wq8[:], wt[:], 1.0, -1.0, Alu.min, Alu.max)
                    nc.sync.dma_start(w2q_dram[k * 128:(k + 1) * 128, c0:c0 + 1024],
                                      wq8[:])

            # ---------------- fc1: h.T = gelu(w1q.T-contraction) ----------------
            KHQ = KH // 4
            for blk in range(NBLK):
                crk = blk // BPC
                coff = (blk % BPC) * nb
                xq_half = []
                for hf in range(4):
                    xqh = xqp.tile([128, KHQ, nb], BF16, tag="xq")
                    nc.sync.dma_start(
                        xqh[:],
                        xqT_full[crk * H + hf * KHQ * 128: crk * H + (hf + 1) * KHQ * 128,
                                 coff:coff + nb].rearrange("(k p) c -> p k c", p=128))
                    xq_half.append(xqh)

                s_row = small.tile([1, nb], F32, tag="srow")
                nc.sync.dma_start(
                    s_row[:], sx_full[blk * nb:(blk + 1) * nb].rearrange("(a f) -> a f", a=1))
                ps_b = psbc.tile([128, nb], F32, tag="psbc")
                nc.tensor.matmul(ps_b[:], lhsT=ones_row[:], rhs=s_row[:],
                                 start=True, stop=True)
                m1_t = small.tile([128, nb], F32, tag="bc")
                nc.vector.tensor_scalar(m1_t[:], ps_b[:], sw1_127_col[:], None, Alu.mult)

                gmax = small.tile([128, nb], FP16, tag="gmax")
                gq_slab = None
                for m in range(MI):
                    ps = psum.tile([128, nb], F32, tag="ps")
                    for k in range(KH):
                        nc.tensor.matmul(
                            ps[:], lhsT=w1sb[:, k, m * 128:(m + 1) * 128],
                            rhs=xq_half[k // KHQ][:, k % KHQ, :],
                            start=(k == 0), stop=(k == KH - 1))
                    g = workA.tile([128, nb], F32, tag="g")
                    nc.vector.tensor_tensor(g[:], ps[:], m1_t[:], Alu.mult)
                    if m % 2 == 0:
                        gq_slab = workA.tile([128, 2, nb], FP16, tag="gq")
                    nc.scalar.activation(gq_slab[:, m % 2, :], g[:], Act.Gelu)
                    gabs = workA.tile([128, nb], FP16, tag="gabs")
                    nc.scalar.activation(gabs[:], gq_slab[:, m % 2, :], Act.Abs)
                    if m == 0:
                        nc.vector.tensor_copy(gmax[:], gabs[:])
                    else:
                        nc.vector.tensor_tensor(gmax[:], gmax[:], gabs[:], Alu.max)
                    if m % 2 == 1:
                        mg = m // 2
                        nc.sync.dma_start(
                            h_dram[mg * 256:(mg + 1) * 256, blk * nb:(blk + 1) * nb]
                            .rearrange("(mi p) c -> p mi c", p=128),
                            gq_slab[:])
                # fold per-token max over partitions: 128 -> 32, transpose, reduce
                ftmp = small.tile([64, nb], FP16, tag="gmt")
                nc.vector.tensor_copy(ftmp[0:64], gmax[64:128])
                nc.vector.tensor_tensor(gmax[0:64], gmax[0:64], ftmp[0:64], Alu.max)
                nc.vector.tensor_copy(ftmp[0:32], gmax[32:64])
                nc.vector.tensor_tensor(gmax[0:32], gmax[0:32], ftmp[0:32], Alu.max)
                gmt = small.tile([32, nb], FP16, tag="gmt")
                nc.vector.transpose(gmt[:], gmax[0:32, :])
                red = small.tile([32, NT32], F32, tag="red")
                nc.vector.tensor_reduce(
                    red[:], gmt[:].rearrange("p (c q) -> p c q", q=32),
                    axis=mybir.AxisListType.X, op=Alu.max)
                nc.sync.dma_start(
                    hmax_in[blk * nb:(blk + 1) * nb].rearrange("(c p) -> p c", p=32),
                    red[:])

            nc.gpsimd.collective_compute(
                "AllReduce", Alu.max, replica_groups=rg,
                ins=[hmax_in[:].opt()], outs=[hmax_out[:].opt()])

            # ---------------- fc2: out[tok, H] with swapped operands ----------
            w2sb = wpool.tile([128, KI, H], FP8, tag="w")
            nc.sync.dma_start(w2sb[:],
                              w2q_dram[:, :].rearrange("(k p) h -> p k h", p=128))

            KIH = KI // 2
            for blk in range(NBLK):
                hl_half = []
                for hf in range(2):
                    hlh = hlp.tile([128, KIH, nb], FP16, tag="hl")
                    nc.sync.dma_start(
                        hlh[:],
                        h_dram[hf * KIH * 128:(hf + 1) * KIH * 128,
                               blk * nb:(blk + 1) * nb]
                        .rearrange("(k p) c -> p k c", p=128))
                    hl_half.append(hlh)

                s_row = small.tile([1, nb], F32, tag="srow")
                nc.sync.dma_start(
                    s_row[:],
                    hmax_out[blk * nb:(blk + 1) * nb].rearrange("(a f) -> a f", a=1))
                nc.vector.tensor_scalar_max(s_row[:], s_row[:], EPS)
                r_row = small.tile([1, nb], F32, tag="rrow")
                nc.vector.reciprocal(r_row[:], s_row[:])
                ps_b = psbc.tile([128, nb], F32, tag="psbc")
                nc.tensor.matmul(ps_b[:], lhsT=ones_row[:], rhs=r_row[:],
                                 start=True, stop=True)
                rq_t = small.tile([128, nb], F32, tag="bc")
                nc.vector.tensor_scalar_mul(rq_t[:], ps_b[:], 127.0)

                # per-token out-scale columns: m2[tok] = sh*sw2/127 (per-partition)
                ps_m2 = psbc.tile([128, 4], F32, tag="psbc")
                for j in range(NTS):
                    nc.tensor.matmul(ps_m2[:, j:j + 1],
                                     lhsT=s_row[0:1, j * 128:(j + 1) * 128],
                                     rhs=sw2_127_sb[:], start=True, stop=True)
                m2cols = small.tile([128, NTS], F32, tag="m2c")
                nc.vector.tensor_copy(m2cols[:], ps_m2[:, 0:NTS])

                hq = hqp.tile([128, KI, nb], BF16, tag="hq")
                for ki in range(KI):
                    ht = workB.tile([128, nb], F32, tag="ht")
                    nc.vector.tensor_tensor(ht[:], hl_half[ki // KIH][:, ki % KIH, :],
                                            rq_t[:], Alu.mult)
                    nc.vector.tensor_scalar(hq[:, ki, :], ht[:], MAGIC, MAGIC,
                                            Alu.add, Alu.subtract)

                for ts_ in range(NTS):
                    for hs in range(NHS):
                        ps = psum.tile([128, nb], F32, tag="ps")
                        for ki in range(KI):
                            nc.tensor.matmul(
                                ps[:], lhsT=hq[:, ki, ts_ * 128:(ts_ + 1) * 128],
                                rhs=w2sb[:, ki, hs * nb:(hs + 1) * nb],
                                start=(ki == 0), stop=(ki == KI - 1))
                        ot = workA.tile([128, nb], FP16, tag="ot")
                        nc.vector.tensor_scalar(ot[:], ps[:], m2cols[:, ts_:ts_ + 1],
                                                None, Alu.mult)
                        nc.sync.dma_start(
                            rs_in[blk][ts_ * 128:(ts_ + 1) * 128, hs * nb:(hs + 1) * nb],
                            ot[:])
                nc.gpsimd.collective_compute(
                    "ReduceScatter", Alu.add, replica_groups=rg,
                    ins=[rs_in[blk][:].opt()], outs=[rs_out[blk][:].opt()])
                nc.sync.dma_start(out_e[blk * TPB:(blk + 1) * TPB, :], rs_out[blk][:, :])

    nc.compile()
    return nc


_PROGRAM_CACHE = {}


def _get_program(key):
    if key not in _PROGRAM_CACHE:
        _PROGRAM_CACHE[key] = build_program(*key)
    return _PROGRAM_CACHE[key]


def make_in_maps(x, W1, W2, ncores=NCORES):
    t, h = x.reshape(-1, x.shape[-1]).shape
    i = W1.shape[0]
    xf = np.ascontiguousarray(x.reshape(t, h), dtype=np.float32)
    ts, isd = t // ncores, i // ncores
    in_maps = []
    for c in range(ncores):
        xs = xf[c * ts:(c + 1) * ts]
        in_maps.append({
            "x": xs,
            "xt": np.ascontiguousarray(xs.T),
            "w1t": np.ascontiguousarray(W1[c * isd:(c + 1) * isd, :].T, dtype=np.float32),
            "w2t": np.ascontiguousarray(W2[:, c * isd:(c + 1) * isd].T, dtype=np.float32),
        })
    return in_maps


def assemble_out(per_core_out, x_shape, t, h, ncores=NCORES, nb=512):
    """per_core_out[c]: [T/ncores, H] fp16, rows = (blk, tok-in-rank-shard).
    token t = blk*nb + c*(nb/ncores) + r."""
    nblk = t // nb
    tpb = nb // ncores
    stacked = np.stack(per_core_out, axis=0).reshape(ncores, nblk, tpb, h)
    out = stacked.transpose(1, 0, 2, 3).reshape(t, h).astype(np.float32)
    return out.reshape(x_shape)


def run(x, W1, W2, trace=False, trace_kwargs=None):
    """Run the distributed kernel on full inputs. Returns (out, BassKernelResults)."""
    t, h = x.reshape(-1, x.shape[-1]).shape
    i = W1.shape[0]
    nc = _get_program((t, h, i, NCORES))
    in_maps = make_in_maps(x, W1, W2)
    res = bass_utils.run_bass_kernel_spmd(
        nc, in_maps, core_ids=list(range(NCORES)), trace=trace,
        **(trace_kwargs or {}),
    )
    out = assemble_out([res.results[c]["out_h"] for c in range(NCORES)],
                       x.shape, t, h)
    return out, res


def kernel(x, W1, W2):
    out, _ = run(x, W1, W2)
    return out


class TimedRunner:
    """Compile once, keep inputs on device, time repeated executions.

    Mirrors bass2jax.run_bass_via_pjrt's multi-core path but persists the
    device-side inputs so repeat calls measure (dispatch + HW execution)
    only, not the host->device staging.
    """

    def __init__(self, nc, in_maps):
        import jax
        import concourse.mybir as mybir_
        from concourse import bass2jax
        from jax.experimental.shard_map import shard_map
        from jax.sharding import Mesh, PartitionSpec, NamedSharding

        bass2jax.install_neuronx_cc_hook()
        n_cores = len(in_maps)
        partition_name = nc.partition_id_tensor.name if nc.partition_id_tensor else None
        in_names, out_names, out_avals = [], [], []
        for alloc in nc.m.functions[0].allocations:
            if not isinstance(alloc, mybir_.MemoryLocationSet):
                continue
            name = alloc.memorylocations[0].name
            if alloc.kind == "ExternalInput":
                if name != partition_name:
                    in_names.append(name)
            elif alloc.kind == "ExternalOutput":
                out_names.append(name)
                out_avals.append(jax.core.ShapedArray(
                    tuple(alloc.tensor_shape), mybir_.dt.np(alloc.dtype)))
        n_params = len(in_names)
        n_outs = len(out_avals)
        all_in_names = list(in_names) + list(out_names)
        if partition_name is not None:
            all_in_names.append(partition_name)
        donate = tuple(range(n_params, n_params + n_outs))

        def _body(*args):
            operands = list(args)
            if partition_name is not None:
                operands.append(bass2jax.partition_id_tensor())
            outs = bass2jax._bass_exec_p.bind(
                *operands,
                out_avals=tuple(out_avals),
                in_names=tuple(all_in_names),
                out_names=tuple(out_names),
                lowering_input_output_aliases=(),
                sim_require_finite=True,
                sim_require_nnan=True,
                nc=nc,
            )
            return tuple(outs)

        devices = jax.devices()[:n_cores]
        mesh = Mesh(np.asarray(devices), ("core",))
        in_specs = (PartitionSpec("core"),) * (n_params + n_outs)
        out_specs = (PartitionSpec("core"),) * n_outs
        self._fn = jax.jit(
            shard_map(_body, mesh=mesh, in_specs=in_specs, out_specs=out_specs,
                      check_rep=False),
            donate_argnums=donate, keep_unused=True,
        )
        sh = NamedSharding(mesh, PartitionSpec("core"))
        concat_in = [
            np.concatenate([np.asarray(in_maps[c][nm]) for c in range(n_cores)], axis=0)
            for nm in in_names
        ]
        self._dev_in = [jax.device_put(a, sh) for a in concat_in]
        self._zero_shapes = [(n_cores * a.shape[0], *a.shape[1:]) for a in out_avals]
        self._zero_dtypes = [a.dtype for a in out_avals]
        self._sh = sh
        self._jax = jax
        self.out_names = out_names
        self.out_avals = out_avals
        self.n_cores = n_cores

    def run_once(self):
        import time
        jax = self._jax
        zeros = [jax.device_put(np.zeros(s, d), self._sh)
                 for s, d in zip(self._zero_shapes, self._zero_dtypes)]
        jax.block_until_ready(zeros)
        t0 = time.perf_counter()
        outs = self._fn(*self._dev_in, *zeros)
        jax.block_until_ready(outs)
        dt = time.perf_counter() - t0
        results = [
            {nm: np.asarray(outs[i]).reshape(self.n_cores, *self.out_avals[i].shape)[c]
             for i, nm in enumerate(self.out_names)}
            for c in range(self.n_cores)
        ]
        return results, dt


# revision 18
# speedup vs baseline: 1.0465x; 1.0078x over previous
"""BitNet Transformer MLP on 8 Trainium2 NeuronCores.

Math (per reference):
  sw1 = max(mean|W1|, EPS); wq1 = clip(round(W1/sw1), -1, 1)
  sx[t] = max(max_h|x[t,h]|, EPS)/127; xq = round(x/sx)      (ints in [-127,127])
  h = gelu((xq @ wq1.T) * sx * sw1)                           (exact erf gelu)
  sh[t] = max(max_i|h[t,i]|, EPS)/127; hq = round(h/sh)
  out = (hq @ wq2.T) * sh * sw2

Sharding (tensor-parallel over the intermediate dim I):
  - tokens T flattened; core c quantizes its T/8 token slice in transposed
    layout, AllGather -> xqT (bf16, exact), chunk-interleaved by rank
  - core c holds W1 rows [c*I/8:(c+1)*I/8] and W2 cols [same I-slice];
    host feeds shards PRE-TRANSPOSED: w1t=[H, I/8], w2t=[I/8, H], xt=[H, T/8]
  - per-tensor weight scales via two 1-float AllReduces (W1 first so fc1's
    weight quantization starts before W2 is even scanned)
  - ternary weights are held in SBUF as fp8e4 (exact for {-1,0,1}); both
    weight matrices stay resident (64 KB/partition each)
  - fc1 computes h.T slice [I/8, T] (PE contracts H; lhsT=weights fp8,
    rhs=xq bf16 -- mixed-dtype matmul is exact here), spills h.T as fp16
  - per-token max|h| partials -> one AllReduce(max) of [T]
  - fc2 uses SWAPPED operands: lhsT=hq.T slice (stationary bf16), rhs=w2q
    (moving fp8) -> partial out [tokens, H]; the per-token scale sh*sw2/127
    is a per-PARTITION scalar folded into the PSUM->SBUF copy, and
    ReduceScatter(add) over the token axis hands core c final tokens
    [blk*nb + c*nb/8 ...) per block -- no post-scale pass, no transpose.
  - host interleaves the 8 token-sharded outputs and casts fp16->f32.

All matmuls are exact: quantized activations are integers <=127 (bf16
exact) and weights are ternary (fp8e4 exact); accumulation is fp32 PSUM.
"""

import numpy as np

import concourse.bass as bass
import concourse.mybir as mybir
import concourse.tile as tile
from concourse import bass_utils, bacc

F32 = mybir.dt.float32
BF16 = mybir.dt.bfloat16
FP16 = mybir.dt.float16
FP8 = mybir.dt.float8e4
MAGIC = 12582912.0  # 1.5*2^23: (v+MAGIC)-MAGIC == round-to-nearest-even, |v|<2^22
EPS = 1e-5
Alu = mybir.AluOpType
Act = mybir.ActivationFunctionType

# full problem config
B, S, H, I = 4, 2048, 4096, 16384
T = B * S
NCORES = 8


def build_program(T=T, H=H, I=I, ncores=NCORES, nb=512):
    TS = T // ncores          # token shard (quant phase)
    IS = I // ncores          # I shard per core
    NBLK = T // nb            # token blocks
    KH = H // 128             # fc1 contraction tiles
    KI = IS // 128            # fc2 contraction tiles
    MI = IS // 128            # fc1 m-tiles (h.T rows / 128)
    NHS = H // nb             # fc2 H column slices
    NTS = nb // 128           # fc2 token sub-tiles per block
    TPB = nb // ncores        # tokens per rank per block (RS shard)
    CH = 512                  # phase-Q free-dim chunk
    BPC = TS // nb            # token blocks per AG rank-chunk
    NT32 = nb // 32
    assert nb % 128 == 0 and TS % nb == 0 and IS % 128 == 0 and nb % ncores == 0

    nc = bacc.Bacc("TRN2", target_bir_lowering=False, debug=False, num_devices=ncores)

    x_e = nc.dram_tensor("x", [TS, H], F32, kind="ExternalInput")
    xt_e = nc.dram_tensor("xt", [H, TS], F32, kind="ExternalInput")
    w1t_e = nc.dram_tensor("w1t", [H, IS], F32, kind="ExternalInput")
    w2t_e = nc.dram_tensor("w2t", [IS, H], F32, kind="ExternalInput")
    out_e = nc.dram_tensor("out_h", [NBLK * TPB, H], FP16, kind="ExternalOutput")

    rg = [list(range(ncores))]

    with tile.TileContext(nc) as tc:
        with (
            tc.tile_pool(name="singles", bufs=1) as singles,
            tc.tile_pool(name="wpool", bufs=1) as wpool,
            tc.tile_pool(name="xqp", bufs=5) as xqp,
            tc.tile_pool(name="hlp", bufs=2) as hlp,
            tc.tile_pool(name="hqp", bufs=2) as hqp,
            tc.tile_pool(name="workA", bufs=2) as workA,
            tc.tile_pool(name="workB", bufs=2) as workB,
            tc.tile_pool(name="small", bufs=2) as small,
            tc.tile_pool(name="psum", bufs=6, space="PSUM") as psum,
            tc.tile_pool(name="psbc", bufs=2, space="PSUM") as psbc,
            tc.tile_pool(name="dram", bufs=1, space="DRAM") as dram,
        ):
            # ---------------- DRAM scratch ----------------
            sx_ag_in = dram.tile([TS], F32, name="sx_ag_in")
            sx_full = dram.tile([T], F32, name="sx_full", addr_space="Shared")
            xq_ag_in = dram.tile([H, TS], BF16, name="xq_ag_in")
            xqT_full = dram.tile([ncores * H, TS], BF16, name="xqT_full",
                                 addr_space="Shared")
            ws1_in = dram.tile([1, 1], F32, name="ws1_in")
            ws1_out = dram.tile([1, 1], F32, name="ws1_out", addr_space="Shared")
            ws2_in = dram.tile([1, 1], F32, name="ws2_in")
            ws2_out = dram.tile([1, 1], F32, name="ws2_out", addr_space="Shared")
            w2q_dram = dram.tile([IS, H], FP8, name="w2q_dram")
            h_dram = dram.tile([IS, T], FP16, name="h_dram")
            hmax_in = dram.tile([T], F32, name="hmax_in")
            hmax_out = [dram.tile([T // 2], F32, name=f"hmax_out_{j}", addr_space="Shared")
                        for j in range(2)]
            rs_in = [dram.tile([nb, H], FP16, name=f"rs_in_{j}") for j in range(NBLK)]
            rs_out = [dram.tile([TPB, H], FP16, name=f"rs_out_{j}")
                      for j in range(NBLK)]

            # ---------------- constants ----------------
            ones_row = singles.tile([1, 128], F32, name="ones_row")
            nc.any.memset(ones_row[:], 1.0)
            ones_col = singles.tile([128, 1], F32, name="ones_col")
            nc.any.memset(ones_col[:], 1.0)

            # ---------------- phase Q: W1 |.| partial sum -> AR1 ----------------
            def abs_sum(src, rows, cols, dst_dram, tag, dma=None):
                dma = dma or nc.sync
                acc = singles.tile([128, 1], F32, name=f"acc_{tag}")
                first = True
                for it in range(rows // 128):
                    for c0 in range(0, cols, 1024):
                        wt = workB.tile([128, 1024], F32, tag="wb")
                        dma.dma_start(wt[:], src[it * 128:(it + 1) * 128,
                                                 c0:c0 + 1024])
                        part = small.tile([128, 1], F32, tag="part")
                        nc.vector.tensor_reduce(part[:], wt[:], axis=mybir.AxisListType.X,
                                                op=Alu.add, apply_absolute_value=True)
                        if first:
                            nc.vector.tensor_copy(acc[:], part[:])
                            first = False
                        else:
                            nc.vector.tensor_tensor(acc[:], acc[:], part[:], Alu.add)
                ps_f = psbc.tile([128, 4], F32, tag="psbc")
                pss = ps_f[0:1, 0:1]
                nc.tensor.matmul(pss, lhsT=acc[:], rhs=ones_col[:], start=True, stop=True)
                ssb = small.tile([1, 1], F32, tag="ssb")
                nc.vector.tensor_copy(ssb[:], pss)
                nc.sync.dma_start(dst_dram[:, :], ssb[:])

            abs_sum(w1t_e, H, IS, ws1_in, "w1")

            # x row maxima (own token shard); DMAs queue after the w1 scan
            for it in range(TS // 128):
                rowmax = small.tile([128, 1], F32, tag="rmax")
                for ch, c0 in enumerate(range(0, H, CH)):
                    xtile = workA.tile([128, CH], F32, tag="wa")
                    nc.sync.dma_start(xtile[:], x_e[it * 128:(it + 1) * 128,
                                                    c0:c0 + CH])
                    part = small.tile([128, 1], F32, tag="part")
                    nc.vector.tensor_reduce(part[:], xtile[:], axis=mybir.AxisListType.X,
                                            op=Alu.max, apply_absolute_value=True)
                    if ch == 0:
                        nc.vector.tensor_scalar_max(rowmax[:], part[:], EPS)
                    else:
                        nc.vector.tensor_tensor(rowmax[:], rowmax[:], part[:], Alu.max)
                nc.sync.dma_start(
                    sx_ag_in[it * 128:(it + 1) * 128].rearrange("(p a) -> p a", a=1),
                    rowmax[:])

            nc.gpsimd.collective_compute(
                "AllReduce", Alu.add, replica_groups=rg,
                ins=[ws1_in[:].opt()], outs=[ws1_out[:].opt()])
            nc.gpsimd.collective_compute(
                "AllGather", Alu.bypass, replica_groups=rg,
                ins=[sx_ag_in[:].opt()], outs=[sx_full[:].opt()])

            # ---------------- phase Q: quantize x (transposed layout) --------
            for c0 in range(0, TS, CH):
                rq_row = small.tile([1, CH], F32, tag="srow")
                nc.sync.dma_start(rq_row[:],
                                  sx_ag_in[c0:c0 + CH].rearrange("(a f) -> a f", a=1))
                nc.vector.reciprocal(rq_row[:], rq_row[:])
                nc.vector.tensor_scalar_mul(rq_row[:], rq_row[:], 127.0)
                ps = psbc.tile([128, CH], F32, tag="psbc")
                nc.tensor.matmul(ps[:], lhsT=ones_row[:], rhs=rq_row[:],
                                 start=True, stop=True)
                rq_bc = small.tile([128, CH], F32, tag="bc")
                nc.vector.tensor_copy(rq_bc[:], ps[:])
                for it in range(H // 128):
                    xtile = workA.tile([128, CH], F32, tag="wa")
                    nc.sync.dma_start(xtile[:, :CH],
                                      xt_e[it * 128:(it + 1) * 128, c0:c0 + CH])
                    nc.vector.tensor_tensor(xtile[:, :CH], xtile[:, :CH], rq_bc[:],
                                            Alu.mult)
                    xqt = workA.tile([128, CH], BF16, tag="xbf")
                    nc.vector.tensor_scalar(xqt[:, :CH], xtile[:, :CH], MAGIC, MAGIC,
                                            Alu.add, Alu.subtract)
                    nc.sync.dma_start(xq_ag_in[it * 128:(it + 1) * 128, c0:c0 + CH],
                                      xqt[:, :CH])

            nc.gpsimd.collective_compute(
                "AllGather", Alu.bypass, replica_groups=rg,
                ins=[xq_ag_in[:].opt()], outs=[xqT_full[:].opt()])

            # ---------------- scale scalars + broadcast columns ----------------
            def bcast_scalar(src_ap, name):
                ps_f = psbc.tile([128, 4], F32, tag="psbc")
                ps = ps_f[:, 0:1]
                nc.tensor.matmul(ps, lhsT=ones_row[:], rhs=src_ap, start=True, stop=True)
                t = singles.tile([128, 1], F32, name=name)
                nc.vector.tensor_copy(t[:], ps)
                return t

            def weight_scale(ws_out_dram, denom, tag):
                sw_sb = singles.tile([1, 1], F32, name=f"sw_{tag}")
                nc.sync.dma_start(sw_sb[:], ws_out_dram[:, :])
                nc.vector.tensor_scalar(sw_sb[:], sw_sb[:], 1.0 / denom, EPS,
                                        Alu.mult, Alu.max)
                rsw_sb = singles.tile([1, 1], F32, name=f"rsw_{tag}")
                nc.vector.reciprocal(rsw_sb[:], sw_sb[:])
                return sw_sb, rsw_sb

            sw1_sb, rsw1_sb = weight_scale(ws1_out, float(I) * H, "w1")
            rsw1_col = bcast_scalar(rsw1_sb[:], "rsw1_col")
            sw1_127_col = bcast_scalar(sw1_sb[:], "sw1_127_col")
            nc.vector.tensor_scalar_mul(sw1_127_col[:], sw1_127_col[:], 1.0 / 127.0)

            # ---------------- quantize W1 -> fp8 resident in SBUF ----------------
            w1sb = wpool.tile([128, KH, IS], FP8, tag="w")
            for k in range(KH):
                for c0 in range(0, IS, 1024):
                    wt = workB.tile([128, 1024], F32, tag="wb")
                    nc.sync.dma_start(wt[:], w1t_e[k * 128:(k + 1) * 128, c0:c0 + 1024])
                    nc.scalar.mul(wt[:], wt[:], rsw1_col[:])
                    nc.vector.tensor_scalar(wt[:], wt[:], MAGIC, MAGIC,
                                            Alu.add, Alu.subtract)
                    nc.vector.tensor_scalar(w1sb[:, k, c0:c0 + 1024], wt[:], 1.0, -1.0,
                                            Alu.min, Alu.max)

            # ---------------- W2 scan/AR/quantize (overlaps fc1) ----------------
            abs_sum(w2t_e, IS, H, ws2_in, "w2", dma=nc.scalar)
            nc.gpsimd.collective_compute(
                "AllReduce", Alu.add, replica_groups=rg,
                ins=[ws2_in[:].opt()], outs=[ws2_out[:].opt()])
            sw2_sb, rsw2_sb = weight_scale(ws2_out, float(I) * H, "w2")
            rsw2_col = bcast_scalar(rsw2_sb[:], "rsw2_col")
            sw2_127_sb = singles.tile([1, 1], F32, name="sw2_127_sb")
            nc.vector.tensor_scalar_mul(sw2_127_sb[:], sw2_sb[:], 1.0 / 127.0)
            for k in range(KI):
                for c0 in range(0, H, 1024):
                    wt = workB.tile([128, 1024], F32, tag="wb")
                    nc.scalar.dma_start(wt[:], w2t_e[k * 128:(k + 1) * 128,
                                                     c0:c0 + 1024])
                    nc.scalar.mul(wt[:], wt[:], rsw2_col[:])
                    nc.vector.tensor_scalar(wt[:], wt[:], MAGIC, MAGIC,
                                            Alu.add, Alu.subtract)
                    wq8 = workB.tile([128, 1024], FP8, tag="wq8")
                    nc.vector.tensor_scalar(wq8[:], wt[:], 1.0, -1.0, Alu.min, Alu.max)
                    nc.scalar.dma_start(w2q_dram[k * 128:(k + 1) * 128, c0:c0 + 1024],
                                        wq8[:])

            # ---------------- fc1: h.T = gelu(w1q.T-contraction) ----------------
            KHQ = KH // 4
            for blk in range(NBLK):
                crk = blk // BPC
                coff = (blk % BPC) * nb
                xq_half = []
                for hf in range(4):
                    xqh = xqp.tile([128, KHQ, nb], BF16, tag="xq")
                    nc.sync.dma_start(
                        xqh[:],
                        xqT_full[crk * H + hf * KHQ * 128: crk * H + (hf + 1) * KHQ * 128,
                                 coff:coff + nb].rearrange("(k p) c -> p k c", p=128))
                    xq_half.append(xqh)

                s_row = small.tile([1, nb], F32, tag="srow")
                nc.sync.dma_start(
                    s_row[:], sx_full[blk * nb:(blk + 1) * nb].rearrange("(a f) -> a f", a=1))
                ps_b = psbc.tile([128, nb], F32, tag="psbc")
                nc.tensor.matmul(ps_b[:], lhsT=ones_row[:], rhs=s_row[:],
                                 start=True, stop=True)
                m1_t = small.tile([128, nb], F32, tag="bc")
                nc.vector.tensor_scalar(m1_t[:], ps_b[:], sw1_127_col[:], None, Alu.mult)

                gmax = small.tile([128, nb], FP16, tag="gmax")
                gq_slab = None
                for m in range(MI):
                    ps = psum.tile([128, nb], F32, tag="ps")
                    for k in range(KH):
                        nc.tensor.matmul(
                            ps[:], lhsT=w1sb[:, k, m * 128:(m + 1) * 128],
                            rhs=xq_half[k // KHQ][:, k % KHQ, :],
                            start=(k == 0), stop=(k == KH - 1))
                    g = workA.tile([128, nb], F32, tag="g")
                    nc.vector.tensor_tensor(g[:], ps[:], m1_t[:], Alu.mult)
                    if m % 2 == 0:
                        gq_slab = workA.tile([128, 2, nb], FP16, tag="gq")
                    nc.scalar.activation(gq_slab[:, m % 2, :], g[:], Act.Gelu)
                    gabs = workA.tile([128, nb], FP16, tag="gabs")
                    nc.scalar.activation(gabs[:], gq_slab[:, m % 2, :], Act.Abs)
                    if m == 0:
                        nc.vector.tensor_copy(gmax[:], gabs[:])
                    else:
                        nc.vector.tensor_tensor(gmax[:], gmax[:], gabs[:], Alu.max)
                    if m % 2 == 1:
                        mg = m // 2
                        nc.sync.dma_start(
                            h_dram[mg * 256:(mg + 1) * 256, blk * nb:(blk + 1) * nb]
                            .rearrange("(mi p) c -> p mi c", p=128),
                            gq_slab[:])
                # fold per-token max over partitions: 128 -> 32, transpose, reduce
                ftmp = small.tile([64, nb], FP16, tag="gmt")
                nc.vector.tensor_copy(ftmp[0:64], gmax[64:128])
                nc.vector.tensor_tensor(gmax[0:64], gmax[0:64], ftmp[0:64], Alu.max)
                nc.vector.tensor_copy(ftmp[0:32], gmax[32:64])
                nc.vector.tensor_tensor(gmax[0:32], gmax[0:32], ftmp[0:32], Alu.max)
                gmt = small.tile([32, nb], FP16, tag="gmt")
                nc.vector.transpose(gmt[:], gmax[0:32, :])
                red = small.tile([32, NT32], F32, tag="red")
                nc.vector.tensor_reduce(
                    red[:], gmt[:].rearrange("p (c q) -> p c q", q=32),
                    axis=mybir.AxisListType.X, op=Alu.max)
                nc.sync.dma_start(
                    hmax_in[blk * nb:(blk + 1) * nb].rearrange("(c p) -> p c", p=32),
                    red[:])
                if blk == NBLK // 2 - 1:
                    nc.gpsimd.collective_compute(
                        "AllReduce", Alu.max, replica_groups=rg,
                        ins=[hmax_in[0:T // 2].opt()], outs=[hmax_out[0][:].opt()])
                elif blk == NBLK - 1:
                    nc.gpsimd.collective_compute(
                        "AllReduce", Alu.max, replica_groups=rg,
                        ins=[hmax_in[T // 2:T].opt()], outs=[hmax_out[1][:].opt()])



            # ---------------- fc2: out[tok, H] with swapped operands ----------
            w2sb = wpool.tile([128, KI, H], FP8, tag="w")
            nc.sync.dma_start(w2sb[:],
                              w2q_dram[:, :].rearrange("(k p) h -> p k h", p=128))

            KIH = KI // 2
            for blk in range(NBLK):
                hl_half = []
                for hf in range(2):
                    hlh = hlp.tile([128, KIH, nb], FP16, tag="hl")
                    nc.sync.dma_start(
                        hlh[:],
                        h_dram[hf * KIH * 128:(hf + 1) * KIH * 128,
                               blk * nb:(blk + 1) * nb]
                        .rearrange("(k p) c -> p k c", p=128))
                    hl_half.append(hlh)

                s_row = small.tile([1, nb], F32, tag="srow")
                hm_half = hmax_out[blk // (NBLK // 2)]
                hm_off = (blk % (NBLK // 2)) * nb
                nc.sync.dma_start(
                    s_row[:],
                    hm_half[hm_off:hm_off + nb].rearrange("(a f) -> a f", a=1))
                nc.vector.tensor_scalar_max(s_row[:], s_row[:], EPS)
                r_row = small.tile([1, nb], F32, tag="rrow")
                nc.vector.reciprocal(r_row[:], s_row[:])
                ps_b = psbc.tile([128, nb], F32, tag="psbc")
                nc.tensor.matmul(ps_b[:], lhsT=ones_row[:], rhs=r_row[:],
                                 start=True, stop=True)
                rq_t = small.tile([128, nb], F32, tag="bc")
                nc.vector.tensor_scalar_mul(rq_t[:], ps_b[:], 127.0)

                # per-token out-scale columns: m2[tok] = sh*sw2/127 (per-partition)
                ps_m2 = psbc.tile([128, 4], F32, tag="psbc")
                for j in range(NTS):
                    nc.tensor.matmul(ps_m2[:, j:j + 1],
                                     lhsT=s_row[0:1, j * 128:(j + 1) * 128],
                                     rhs=sw2_127_sb[:], start=True, stop=True)
                m2cols = small.tile([128, NTS], F32, tag="m2c")
                nc.vector.tensor_copy(m2cols[:], ps_m2[:, 0:NTS])

                hq = hqp.tile([128, KI, nb], BF16, tag="hq")
                for ki in range(KI):
                    ht = workB.tile([128, nb], F32, tag="ht")
                    nc.vector.tensor_tensor(ht[:], hl_half[ki // KIH][:, ki % KIH, :],
                                            rq_t[:], Alu.mult)
                    nc.vector.tensor_scalar(hq[:, ki, :], ht[:], MAGIC, MAGIC,
                                            Alu.add, Alu.subtract)

                for ts_ in range(NTS):
                    for hs in range(NHS):
                        ps = psum.tile([128, nb], F32, tag="ps")
                        for ki in range(KI):
                            nc.tensor.matmul(
                                ps[:], lhsT=hq[:, ki, ts_ * 128:(ts_ + 1) * 128],
                                rhs=w2sb[:, ki, hs * nb:(hs + 1) * nb],
                                start=(ki == 0), stop=(ki == KI - 1))
                        ot = workA.tile([128, nb], FP16, tag="ot")
                        nc.vector.tensor_scalar(ot[:], ps[:], m2cols[:, ts_:ts_ + 1],
                                                None, Alu.mult)
                        nc.sync.dma_start(
                            rs_in[blk][ts_ * 128:(ts_ + 1) * 128, hs * nb:(hs + 1) * nb],
                            ot[:])
                nc.gpsimd.collective_compute(
                    "ReduceScatter", Alu.add, replica_groups=rg,
                    ins=[rs_in[blk][:].opt()], outs=[rs_out[blk][:].opt()])
                nc.sync.dma_start(out_e[blk * TPB:(blk + 1) * TPB, :], rs_out[blk][:, :])

    nc.compile()
    return nc


_PROGRAM_CACHE = {}


def _get_program(key):
    if key not in _PROGRAM_CACHE:
        _PROGRAM_CACHE[key] = build_program(*key)
    return _PROGRAM_CACHE[key]


def make_in_maps(x, W1, W2, ncores=NCORES):
    t, h = x.reshape(-1, x.shape[-1]).shape
    i = W1.shape[0]
    xf = np.ascontiguousarray(x.reshape(t, h), dtype=np.float32)
    ts, isd = t // ncores, i // ncores
    in_maps = []
    for c in range(ncores):
        xs = xf[c * ts:(c + 1) * ts]
        in_maps.append({
            "x": xs,
            "xt": np.ascontiguousarray(xs.T),
            "w1t": np.ascontiguousarray(W1[c * isd:(c + 1) * isd, :].T, dtype=np.float32),
            "w2t": np.ascontiguousarray(W2[:, c * isd:(c + 1) * isd].T, dtype=np.float32),
        })
    return in_maps


def assemble_out(per_core_out, x_shape, t, h, ncores=NCORES, nb=512):
    """per_core_out[c]: [T/ncores, H] fp16, rows = (blk, tok-in-rank-shard).
    token t = blk*nb + c*(nb/ncores) + r."""
    nblk = t // nb
    tpb = nb // ncores
    stacked = np.stack(per_core_out, axis=0).reshape(ncores, nblk, tpb, h)
    out = stacked.transpose(1, 0, 2, 3).reshape(t, h).astype(np.float32)
    return out.reshape(x_shape)


def run(x, W1, W2, trace=False, trace_kwargs=None):
    """Run the distributed kernel on full inputs. Returns (out, BassKernelResults)."""
    t, h = x.reshape(-1, x.shape[-1]).shape
    i = W1.shape[0]
    nc = _get_program((t, h, i, NCORES))
    in_maps = make_in_maps(x, W1, W2)
    res = bass_utils.run_bass_kernel_spmd(
        nc, in_maps, core_ids=list(range(NCORES)), trace=trace,
        **(trace_kwargs or {}),
    )
    out = assemble_out([res.results[c]["out_h"] for c in range(NCORES)],
                       x.shape, t, h)
    return out, res


def kernel(x, W1, W2):
    out, _ = run(x, W1, W2)
    return out


class TimedRunner:
    """Compile once, keep inputs on device, time repeated executions.

    Mirrors bass2jax.run_bass_via_pjrt's multi-core path but persists the
    device-side inputs so repeat calls measure (dispatch + HW execution)
    only, not the host->device staging.
    """

    def __init__(self, nc, in_maps):
        import jax
        import concourse.mybir as mybir_
        from concourse import bass2jax
        from jax.experimental.shard_map import shard_map
        from jax.sharding import Mesh, PartitionSpec, NamedSharding

        bass2jax.install_neuronx_cc_hook()
        n_cores = len(in_maps)
        partition_name = nc.partition_id_tensor.name if nc.partition_id_tensor else None
        in_names, out_names, out_avals = [], [], []
        for alloc in nc.m.functions[0].allocations:
            if not isinstance(alloc, mybir_.MemoryLocationSet):
                continue
            name = alloc.memorylocations[0].name
            if alloc.kind == "ExternalInput":
                if name != partition_name:
                    in_names.append(name)
            elif alloc.kind == "ExternalOutput":
                out_names.append(name)
                out_avals.append(jax.core.ShapedArray(
                    tuple(alloc.tensor_shape), mybir_.dt.np(alloc.dtype)))
        n_params = len(in_names)
        n_outs = len(out_avals)
        all_in_names = list(in_names) + list(out_names)
        if partition_name is not None:
            all_in_names.append(partition_name)
        donate = tuple(range(n_params, n_params + n_outs))

        def _body(*args):
            operands = list(args)
            if partition_name is not None:
                operands.append(bass2jax.partition_id_tensor())
            outs = bass2jax._bass_exec_p.bind(
                *operands,
                out_avals=tuple(out_avals),
                in_names=tuple(all_in_names),
                out_names=tuple(out_names),
                lowering_input_output_aliases=(),
                sim_require_finite=True,
                sim_require_nnan=True,
                nc=nc,
            )
            return tuple(outs)

        devices = jax.devices()[:n_cores]
        mesh = Mesh(np.asarray(devices), ("core",))
        in_specs = (PartitionSpec("core"),) * (n_params + n_outs)
        out_specs = (PartitionSpec("core"),) * n_outs
        self._fn = jax.jit(
            shard_map(_body, mesh=mesh, in_specs=in_specs, out_specs=out_specs,
                      check_rep=False),
            donate_argnums=donate, keep_unused=True,
        )
        sh = NamedSharding(mesh, PartitionSpec("core"))
        concat_in = [
            np.concatenate([np.asarray(in_maps[c][nm]) for c in range(n_cores)], axis=0)
            for nm in in_names
        ]
        self._dev_in = [jax.device_put(a, sh) for a in concat_in]
        self._zero_shapes = [(n_cores * a.shape[0], *a.shape[1:]) for a in out_avals]
        self._zero_dtypes = [a.dtype for a in out_avals]
        self._sh = sh
        self._jax = jax
        self.out_names = out_names
        self.out_avals = out_avals
        self.n_cores = n_cores

    def run_once(self):
        import time
        jax = self._jax
        zeros = [jax.device_put(np.zeros(s, d), self._sh)
                 for s, d in zip(self._zero_shapes, self._zero_dtypes)]
        jax.block_until_ready(zeros)
        t0 = time.perf_counter()
        outs = self._fn(*self._dev_in, *zeros)
        jax.block_until_ready(outs)
        dt = time.perf_counter() - t0
        results = [
            {nm: np.asarray(outs[i]).reshape(self.n_cores, *self.out_avals[i].shape)[c]
             for i, nm in enumerate(self.out_names)}
            for c in range(self.n_cores)
        ]
        return results, dt
